# revision 14
# baseline (speedup 1.0000x reference)
"""Trainium2 Bass kernel for nn_EncoderLayer_31825707664096.

Gemma-style encoder layer (RMSNorm + GQA attention w/ QK-norm + RoPE + GeGLU
MLP), batch=1, seq=2048, hidden=768, 3 heads x 256 head_dim, 1 KV head,
inter=1152, fp32.

Strategy: sequence-parallel over 8 cores, no collectives (cross-core sync
costs ~90us of launch skew here). Each core owns 256 query rows and
recomputes full K/V, streaming the hidden state through SBUF in 512-column
slices so compute starts ~3us in. All activations are feature-major.

Precision tiers: q/k score path float32r (TF32-like, full PE rate at
free>=256); exp output u and V are bf16 (linear averaging errors only);
O-projection and MLP run bf16 weights/activations. PSUM always fp32;
residual path fp32.

Folds (host or on-chip, all exact):
- (1+ln_in_w) into wq/wk/wv rows; (1+ln_pre_ffn_w) into wg/wu rows
- (1+ln_post_attn_w) into wo columns; the rmsnorm stats then reduce with
  1/(1+w)^2 weights so the rstd sees unscaled values. Same for wd.
- input-norm rstd (rin) cancels in q/k norm; for V it rides the PSUM->SBUF
  copy as a per-partition scale (column stats via transposed PE reduction)
- k-norm rstd (ck) is the per-partition scale of the softmax exp
- softmax runs unnormalized with a constant shift exp(s - C); the per-query
  normalizer is applied to the attention output (reciprocal + broadcast)
"""

from contextlib import ExitStack

import numpy as np
import ml_dtypes

import concourse.mybir as mybir
import concourse.tile as tile
from concourse import bacc
from concourse.bass_utils import run_bass_kernel_spmd

P = 128
S = 2048          # sequence length
H = 768           # hidden
D = 256           # head dim (also total KV width)
NH = 3            # query heads
I = 1152          # mlp intermediate
NC = 8            # cores
SL = S // NC      # 256 query rows per core
SC = SL // P      # 2
HC = H // P       # 6
DC = D // P       # 2
IC = I // P       # 9
KC = S // P       # 16 key chunks
NSL = S // 512    # 4 512-wide slices
EPS = 1e-6
C_SHIFT = 30.0    # exp(s - C_SHIFT): keeps unnormalized softmax in fp32 range

F32 = mybir.dt.float32
F32R = mybir.dt.float32r
BF16 = mybir.dt.bfloat16
MUL = mybir.AluOpType.mult
AF = mybir.ActivationFunctionType

_CACHED = {}


def _build():
    nc = bacc.Bacc("TRN2", target_bir_lowering=False, debug=False, num_devices=NC)

    # ---- DRAM I/O ----
    ht = nc.dram_tensor("ht", [P, NSL, HC, 512], F32R, kind="ExternalInput").ap()
    hq = nc.dram_tensor("hq", [P, HC, SL], F32R, kind="ExternalInput").ap()
    cost = nc.dram_tensor("cost", [P, NSL, DC, 512], F32, kind="ExternalInput").ap()
    sint = nc.dram_tensor("sint", [P, NSL, DC, 512], F32, kind="ExternalInput").ap()
    cosq = nc.dram_tensor("cosq", [P, DC, SL], F32, kind="ExternalInput").ap()
    sinq = nc.dram_tensor("sinq", [P, DC, SL], F32, kind="ExternalInput").ap()
    wqt = nc.dram_tensor("wqt", [P, HC, H], F32R, kind="ExternalInput").ap()
    wkt = nc.dram_tensor("wkt", [P, HC, D], F32R, kind="ExternalInput").ap()
    wvt = nc.dram_tensor("wvt", [P, HC, D], F32R, kind="ExternalInput").ap()
    wot = nc.dram_tensor("wot", [P, HC, H], BF16, kind="ExternalInput").ap()
    wgt = nc.dram_tensor("wgt", [P, HC, I], BF16, kind="ExternalInput").ap()
    wut = nc.dram_tensor("wut", [P, HC, I], BF16, kind="ExternalInput").ap()
    wdt = nc.dram_tensor("wdt", [P, IC, H], BF16, kind="ExternalInput").ap()
    qw1 = nc.dram_tensor("qw1", [P, DC], F32, kind="ExternalInput").ap()   # 1+q_norm_w
    kw1 = nc.dram_tensor("kw1", [P, DC], F32, kind="ExternalInput").ap()   # 1+k_norm_w
    wai = nc.dram_tensor("wai", [P, HC], BF16, kind="ExternalInput").ap()  # (1+ln_post_attn)^-2
    wfi = nc.dram_tensor("wfi", [P, HC], BF16, kind="ExternalInput").ap()  # (1+ln_post_ffn)^-2
    outt = nc.dram_tensor("outt", [P, HC, SL], F32, kind="ExternalOutput").ap()

    def cp(ap2d):  # [(c p), x] -> [p, c, x]
        return ap2d.rearrange("(c p) x -> p c x", p=P)

    def f32(ap):
        return ap.bitcast(F32)

    with tile.TileContext(nc) as tc:
        with (
            tc.tile_pool(name="persist", bufs=1) as pp,
            tc.tile_pool(name="t1", bufs=2) as t1,
            tc.tile_pool(name="pmm", bufs=3, space="PSUM") as pmm,
            tc.tile_pool(name="pst", bufs=1, space="PSUM") as pst,
        ):
            # ---- small constants ----
            ones = pp.tile([P, 1], F32, tag="ones")
            nc.vector.memset(ones[:], 1.0)
            ones_r = ones[:].bitcast(F32R)
            ones_b = pp.tile([P, 1], BF16, tag="ones_b")
            nc.vector.memset(ones_b[:], 1.0)
            ones2b = pp.tile([P, 2], BF16, tag="ones2b")
            nc.vector.memset(ones2b[:], 1.0)
            eps128 = pp.tile([P, 1], F32, tag="eps128")
            nc.vector.memset(eps128[:], EPS)
            biasC = pp.tile([P, 1], F32, tag="biasC")
            nc.vector.memset(biasC[:], -C_SHIFT)
            qw1_sb = pp.tile([P, DC], F32, tag="qw1")
            nc.sync.dma_start(qw1_sb[:], qw1)
            kw1_sb = pp.tile([P, DC], F32, tag="kw1")
            nc.sync.dma_start(kw1_sb[:], kw1)
            wai_sb = pp.tile([P, HC], BF16, tag="wai")
            nc.sync.dma_start(wai_sb[:], wai)
            wfi_sb = pp.tile([P, HC], BF16, tag="wfi")
            nc.sync.dma_start(wfi_sb[:], wfi)

            # ---- startup-critical loads ----
            hq_sb = pp.tile([P, HC, SL], F32R, tag="hq")
            nc.sync.dma_start(hq_sb[:], hq)
            es = ExitStack()
            wp1 = es.enter_context(tc.tile_pool(name="wp1", bufs=1))
            wk_sb = wp1.tile([P, HC, D], F32R, tag="wk")
            nc.scalar.dma_start(wk_sb[:], wkt)
            wv_sb = wp1.tile([P, HC, D], F32R, tag="wv")
            nc.scalar.dma_start(wv_sb[:], wvt)

            # persistent activations
            qt_f = pp.tile([P, HC, SL], F32R, tag="qtf")
            kt_sb = pp.tile([P, DC, S], F32R, tag="ktf")
            v_sb = pp.tile([P, KC, D], BF16, tag="v")
            at_f = pp.tile([P, DC, NH, SL], BF16, tag="atf")
            h2 = pp.tile([P, HC, SL], F32, tag="h2")
            h2n = pp.tile([P, HC, SL], BF16, tag="h2n")
            rin_col = pp.tile([P, KC], F32, tag="rin")
            ck_col = pp.tile([P, KC], F32, tag="ck")

            # =====================================================
            # K + stats over full S, streamed in 512-col slices
            # =====================================================
            esl = ExitStack()
            slp = esl.enter_context(tc.tile_pool(name="slp", bufs=2))
            psc = esl.enter_context(tc.tile_pool(name="psc", bufs=1, space="PSUM"))
            iss_t = psc.tile([P, KC, 2], F32, tag="ips", name="iss")
            ck_t = psc.tile([P, KC, 2], F32, tag="cps", name="cks")
            iss_ps = iss_t[:]
            ck_ps = ck_t[:]

            for sl in range(NSL):
                sls = slice(sl * 512, (sl + 1) * 512)
                hsl = slp.tile([P, HC, 512], F32R, tag="hsl", name=f"hsl{sl}")
                nc.sync.dma_start(hsl[:], ht[:, sl, :, :])
                cs = slp.tile([P, DC, 512], F32, tag="cs", name=f"cs{sl}")
                nc.scalar.dma_start(cs[:], cost[:, sl, :, :])
                sn = slp.tile([P, DC, 512], F32, tag="sn", name=f"sn{sl}")
                nc.scalar.dma_start(sn[:], sint[:, sl, :, :])

                # K projection for this slice
                pk = [psc.tile([P, 512], F32, tag="pk", name=f"pk{sl}_{d_}", bufs=2)
                      for d_ in range(DC)]
                for d in range(DC):
                    for kc in range(HC):
                        nc.tensor.matmul(
                            pk[d][:],
                            wk_sb[:, kc, d * P:(d + 1) * P],
                            hsl[:, kc, :],
                            start=(kc == 0), stop=(kc == HC - 1),
                        )
                # k-norm stats (column orientation)
                ksq = []
                for d in range(DC):
                    sq = t1.tile([P, 512], BF16, tag="ksq", name=f"ksq{sl}_{d}")
                    nc.scalar.activation(sq[:], pk[d][:], AF.Square)
                    ksq.append(sq)
                for c4 in range(4):
                    ksc = 4 * sl + c4
                    for d in range(DC):
                        nc.tensor.matmul(
                            ck_ps[:, ksc, :],
                            ksq[d][:, c4 * P:(c4 + 1) * P],
                            ones2b[:],
                            start=(d == 0), stop=(d == DC - 1),
                        )
                # rope -> kt
                t0 = t1.tile([P, 512], F32, tag="rA", name=f"krA{sl}")
                tb = t1.tile([P, 512], F32, tag="rB", name=f"krB{sl}")
                nc.vector.scalar_tensor_tensor(
                    t0[:], pk[0][:], kw1_sb[:, 0:1], cs[:, 0, :], MUL, MUL)
                nc.vector.scalar_tensor_tensor(
                    tb[:], pk[1][:], kw1_sb[:, 1:2], sn[:, 0, :], MUL, MUL)
                nc.vector.tensor_sub(kt_sb[:, 0, sls], t0[:], tb[:])
                t2 = t1.tile([P, 512], F32, tag="rA", name=f"krC{sl}")
                t3 = t1.tile([P, 512], F32, tag="rB", name=f"krD{sl}")
                nc.vector.scalar_tensor_tensor(
                    t2[:], pk[1][:], kw1_sb[:, 1:2], cs[:, 1, :], MUL, MUL)
                nc.vector.scalar_tensor_tensor(
                    t3[:], pk[0][:], kw1_sb[:, 0:1], sn[:, 1, :], MUL, MUL)
                nc.vector.tensor_add(kt_sb[:, 1, sls], t2[:], t3[:])

                # input sumsq (column orientation) for V scale
                isq_t = []
                for kc in range(HC):
                    isq = t1.tile([P, 512], BF16, tag="isq", name=f"isq{sl}_{kc}",
                                  bufs=HC)
                    nc.vector.tensor_mul(isq[:], f32(hsl[:, kc, :]),
                                         f32(hsl[:, kc, :]))
                    isq_t.append(isq)
                for c4 in range(4):
                    for kc in range(HC):
                        nc.tensor.matmul(
                            iss_ps[:, 4 * sl + c4, :],
                            isq_t[kc][:, c4 * P:(c4 + 1) * P],
                            ones2b[:],
                            start=(kc == 0), stop=(kc == HC - 1),
                        )

                # V projection for this slice (raw; rin applied afterwards)
                for c4 in range(4):
                    sc = 4 * sl + c4
                    pv = pmm.tile([P, D], F32, tag="mm", name=f"pv{sc}")
                    for kc in range(HC):
                        nc.tensor.matmul(
                            pv[:],
                            hsl[:, kc, c4 * P:(c4 + 1) * P],
                            wv_sb[:, kc, :],
                            start=(kc == 0), stop=(kc == HC - 1),
                        )
                    nc.scalar.copy(v_sb[:, sc, :], pv[:])

            # rstd columns: rin [P, KC], ck [P, KC]
            nc.scalar.activation(
                rin_col[:],
                iss_ps[:, :, 0:1].rearrange("p a b -> p (a b)"),
                AF.Sqrt, bias=eps128[:], scale=1.0 / H)
            nc.vector.reciprocal_approx_fast(rin_col[:], rin_col[:])
            nc.scalar.activation(
                ck_col[:],
                ck_ps[:, :, 0:1].rearrange("p a b -> p (a b)"),
                AF.Sqrt, bias=eps128[:], scale=1.0 / D)
            nc.vector.reciprocal_approx_fast(ck_col[:], ck_col[:])

            esl.close()  # free slice ring + slice psum

            # =====================================================
            # Q projection + q-norm + rope
            # =====================================================
            cosq_sb = pp.tile([P, DC, SL], F32, tag="cosq")
            nc.sync.dma_start(cosq_sb[:], cosq)
            sinq_sb = pp.tile([P, DC, SL], F32, tag="sinq")
            nc.sync.dma_start(sinq_sb[:], sinq)
            wq_sb = wp1.tile([P, HC, H], F32R, tag="wq")
            nc.sync.dma_start(wq_sb[:], wqt)
            wo_sb = pp.tile([P, HC, H], BF16, tag="wo")
            nc.sync.dma_start(wo_sb[:], wot)
            wg_sb = pp.tile([P, HC, I], BF16, tag="wg")
            nc.sync.dma_start(wg_sb[:], wgt)
            wu_sb = pp.tile([P, HC, I], BF16, tag="wu")
            nc.sync.dma_start(wu_sb[:], wut)
            wd_sb = pp.tile([P, IC, H], BF16, tag="wd")
            nc.sync.dma_start(wd_sb[:], wdt)

            for h in range(NH):
                pq = [pmm.tile([P, SL], F32, tag="mm", name=f"pq{h}_{d_}")
                      for d_ in range(DC)]
                for d in range(DC):
                    oc = 2 * h + d
                    for kc in range(HC):
                        nc.tensor.matmul(
                            pq[d][:],
                            wq_sb[:, kc, oc * P:(oc + 1) * P],
                            hq_sb[:, kc, :],
                            start=(kc == 0), stop=(kc == HC - 1),
                        )
                qss = pst.tile([1, SL], F32, tag="st1", name=f"qss{h}")
                for d in range(DC):
                    sq = t1.tile([P, SL], F32R, tag="sq", name=f"qsq{h}_{d}")
                    nc.scalar.activation(sq[:], pq[d][:], AF.Square)
                    nc.tensor.matmul(qss[:], ones_r, sq[:],
                                     start=(d == 0), stop=(d == DC - 1))
                cq_row = t1.tile([1, SL], F32, tag="row", name=f"cqr{h}")
                nc.scalar.activation(cq_row[:], qss[:], AF.Sqrt,
                                     bias=eps128[0:1, :], scale=1.0 / D)
                nc.vector.reciprocal_approx_fast(cq_row[:], cq_row[:])
                cq_b = t1.tile([P, SL], F32, tag="bcast", name=f"cqb{h}")
                nc.gpsimd.partition_broadcast(cq_b[:], cq_row[:], channels=P)
                t0 = t1.tile([P, SL], F32, tag="rA", name=f"rA{h}")
                tb = t1.tile([P, SL], F32, tag="rB", name=f"rB{h}")
                nc.vector.scalar_tensor_tensor(
                    t0[:], pq[0][:], qw1_sb[:, 0:1], cosq_sb[:, 0, :], MUL, MUL)
                nc.vector.scalar_tensor_tensor(
                    tb[:], pq[1][:], qw1_sb[:, 1:2], sinq_sb[:, 0, :], MUL, MUL)
                nc.vector.tensor_sub(t0[:], t0[:], tb[:])
                nc.vector.tensor_mul(qt_f[:, 2 * h, :], t0[:], cq_b[:])
                t2 = t1.tile([P, SL], F32, tag="rA", name=f"rC{h}")
                t3 = t1.tile([P, SL], F32, tag="rB", name=f"rD{h}")
                nc.vector.scalar_tensor_tensor(
                    t2[:], pq[1][:], qw1_sb[:, 1:2], cosq_sb[:, 1, :], MUL, MUL)
                nc.vector.scalar_tensor_tensor(
                    t3[:], pq[0][:], qw1_sb[:, 0:1], sinq_sb[:, 1, :], MUL, MUL)
                nc.vector.tensor_add(t2[:], t2[:], t3[:])
                nc.vector.tensor_mul(qt_f[:, 2 * h + 1, :], t2[:], cq_b[:])

            es.close()  # free wq/wk/wv

            # =====================================================
            # attention: scoresT -> exp(scale=ck) -> den -> A^T V
            # =====================================================
            with tc.tile_pool(name="t2", bufs=2) as t2p:
                u3 = t2p.tile([P, NH, KC, SL], BF16, tag="u3", name="u3", bufs=1)
                den_b = t2p.tile([P, NH, SL], F32, tag="denb", name="den_b", bufs=1)
                esa = ExitStack()
                psa = esa.enter_context(
                    tc.tile_pool(name="psa", bufs=1, space="PSUM"))
                # heads 0+1 paired into 512-wide matmuls; head 2 narrow
                for ksc in range(KC):
                    ps2 = psa.tile([P, 2, SL], F32, tag="mm2", name=f"ps2_{ksc}",
                                   bufs=2)
                    for d in range(DC):
                        nc.tensor.matmul(
                            ps2[:],
                            kt_sb[:, d, ksc * P:(ksc + 1) * P],
                            qt_f[:, d:3 + d:2, :],
                            start=(d == 0), stop=(d == DC - 1),
                        )
                    nc.scalar.activation(u3[:, 0:2, ksc, :], ps2[:], AF.Exp,
                                         bias=biasC[:],
                                         scale=ck_col[:, ksc:ksc + 1])
                    ps_ = pmm.tile([P, SL], F32, tag="mm", name=f"ps{ksc}")
                    for d in range(DC):
                        nc.tensor.matmul(
                            ps_[:],
                            kt_sb[:, d, ksc * P:(ksc + 1) * P],
                            qt_f[:, 4 + d, :],
                            start=(d == 0), stop=(d == DC - 1),
                        )
                    nc.scalar.activation(u3[:, 2, ksc, :], ps_[:], AF.Exp,
                                         bias=biasC[:],
                                         scale=ck_col[:, ksc:ksc + 1])

                # apply rin to V (off the exp critical path, on DVE)
                for sc in range(KC):
                    nc.vector.tensor_scalar_mul(v_sb[:, sc, :], v_sb[:, sc, :],
                                                rin_col[:, sc:sc + 1])

                den2 = psa.tile([1, 2, SL], F32, tag="dn2", name="den01")
                for ksc in range(KC):
                    nc.tensor.matmul(
                        den2[:], ones_b[:], u3[:, 0:2, ksc, :],
                        start=(ksc == 0), stop=(ksc == KC - 1))
                den2_row = t1.tile([1, 2, SL], F32, tag="row2", name="denr01")
                nc.vector.reciprocal_approx_fast(
                    den2_row[:].rearrange("o a s -> o (a s)"),
                    den2[:].rearrange("o a s -> o (a s)"))
                for h in range(2):
                    nc.gpsimd.partition_broadcast(den_b[:, h, :],
                                                  den2_row[:, h, :], channels=P)
                den = pst.tile([1, SL], F32, tag="st1", name="den2")
                for ksc in range(KC):
                    nc.tensor.matmul(den[:], ones_b[:], u3[:, 2, ksc, :],
                                     start=(ksc == 0), stop=(ksc == KC - 1))
                den_row = t1.tile([1, SL], F32, tag="row", name="denr2")
                nc.vector.reciprocal_approx_fast(den_row[:], den[:])
                nc.gpsimd.partition_broadcast(den_b[:, 2, :], den_row[:],
                                              channels=P)

                for d in range(DC):
                    pn2 = psa.tile([P, 2, SL], F32, tag="mm2", name=f"pn2_{d}",
                                   bufs=2)
                    for ksc in range(KC):
                        nc.tensor.matmul(
                            pn2[:],
                            v_sb[:, ksc, d * P:(d + 1) * P],
                            u3[:, 0:2, ksc, :],
                            start=(ksc == 0), stop=(ksc == KC - 1),
                        )
                    nc.vector.tensor_mul(at_f[:, d, 0:2, :], pn2[:],
                                         den_b[:, 0:2, :])
                    pn = pmm.tile([P, SL], F32, tag="mm", name=f"pn{d}")
                    for ksc in range(KC):
                        nc.tensor.matmul(
                            pn[:],
                            v_sb[:, ksc, d * P:(d + 1) * P],
                            u3[:, 2, ksc, :],
                            start=(ksc == 0), stop=(ksc == KC - 1),
                        )
                    nc.vector.tensor_mul(at_f[:, d, 2, :], pn[:],
                                         den_b[:, 2, :])
                esa.close()
                at_v = at_f[:].rearrange("p d h s -> p (d h) s")

                # =====================================================
                # wo projection + post-attn rmsnorm + residual
                # =====================================================
                with tc.tile_pool(name="pho", bufs=1, space="PSUM") as pho:
                    ppo2 = [pho.tile([P, 2, SL], F32, tag=f"po{j}", name=f"pp{j}")
                            for j in range(HC // 2)]
                    ppo = [ppo2[j][:, i, :] for j in range(HC // 2) for i in range(2)]
                    pss = pst.tile([1, SL], F32, tag="st1", name="pss")
                    for hc in range(HC):
                        for oc in range(HC):
                            nc.tensor.matmul(
                                ppo[hc],
                                wo_sb[:, oc, hc * P:(hc + 1) * P],
                                at_v[:, oc, :],
                                start=(oc == 0), stop=(oc == HC - 1),
                            )
                        sq = t1.tile([P, SL], BF16, tag="sqb", name=f"psq{hc}")
                        nc.scalar.activation(sq[:], ppo[hc], AF.Square)
                        nc.tensor.matmul(pss[:], wai_sb[:, hc:hc + 1], sq[:],
                                         start=(hc == 0), stop=(hc == HC - 1))
                    ra_row = t1.tile([1, SL], F32, tag="row", name="ra_row")
                    nc.scalar.activation(ra_row[:], pss[:], AF.Sqrt,
                                         bias=eps128[0:1, :], scale=1.0 / H)
                    nc.vector.reciprocal_approx_fast(ra_row[:], ra_row[:])
                    ra_b = t1.tile([P, SL], F32, tag="bcast", name="ra_b")
                    nc.gpsimd.partition_broadcast(ra_b[:], ra_row[:], channels=P)
                    for hc in range(HC):
                        tm = t1.tile([P, SL], F32, tag="htmp", name=f"hm{hc}")
                        nc.vector.tensor_mul(tm[:], ppo[hc], ra_b[:])
                        nc.vector.tensor_add(h2[:, hc, :], tm[:],
                                             f32(hq_sb[:, hc, :]))

                # =====================================================
                # pre-FFN rmsnorm
                # =====================================================
                fss = pst.tile([1, SL], F32, tag="st1", name="fss")
                for hc in range(HC):
                    sq = t1.tile([P, SL], BF16, tag="sqb", name=f"fsq{hc}")
                    nc.vector.tensor_mul(sq[:], h2[:, hc, :], h2[:, hc, :])
                    nc.tensor.matmul(fss[:], ones_b[:], sq[:],
                                     start=(hc == 0), stop=(hc == HC - 1))
                r2_row = t1.tile([1, SL], F32, tag="row", name="r2_row")
                nc.scalar.activation(r2_row[:], fss[:], AF.Sqrt,
                                     bias=eps128[0:1, :], scale=1.0 / H)
                nc.vector.reciprocal_approx_fast(r2_row[:], r2_row[:])
                r2_b = t1.tile([P, SL], F32, tag="bcast", name="r2_b")
                nc.gpsimd.partition_broadcast(r2_b[:], r2_row[:], channels=P)
                for hc in range(HC):
                    nc.vector.tensor_mul(h2n[:, hc, :], h2[:, hc, :], r2_b[:])

                # =====================================================
                # MLP: gate/up -> gelu_tanh * up -> down + post-ffn norm
                # =====================================================
                gall = t2p.tile([P, IC, SL], BF16, tag="gall", name="gall", bufs=1)
                act = t2p.tile([P, IC, SL], BF16, tag="act", name="act", bufs=1)
                for ic in range(IC):
                    pg = pmm.tile([P, SL], F32, tag="mm", name=f"pg{ic}")
                    for kc in range(HC):
                        nc.tensor.matmul(pg[:], wg_sb[:, kc, ic * P:(ic + 1) * P],
                                         h2n[:, kc, :],
                                         start=(kc == 0), stop=(kc == HC - 1))
                    nc.scalar.activation(gall[:, ic, :], pg[:], AF.Gelu_apprx_tanh)
                    pu = pmm.tile([P, SL], F32, tag="mm", name=f"pu{ic}")
                    for kc in range(HC):
                        nc.tensor.matmul(pu[:], wu_sb[:, kc, ic * P:(ic + 1) * P],
                                         h2n[:, kc, :],
                                         start=(kc == 0), stop=(kc == HC - 1))
                    nc.vector.tensor_mul(act[:, ic, :], gall[:, ic, :], pu[:])

                with tc.tile_pool(name="phd", bufs=1, space="PSUM") as phd:
                    pm2 = [phd.tile([P, 2, SL], F32, tag=f"md{j}", name=f"pm{j}")
                           for j in range(HC // 2)]
                    pm = [pm2[j][:, i, :] for j in range(HC // 2) for i in range(2)]
                    mss = pst.tile([1, SL], F32, tag="st1", name="mss")
                    for hc in range(HC):
                        for ic in range(IC):
                            nc.tensor.matmul(pm[hc],
                                             wd_sb[:, ic, hc * P:(hc + 1) * P],
                                             act[:, ic, :],
                                             start=(ic == 0), stop=(ic == IC - 1))
                        sq = t1.tile([P, SL], BF16, tag="sqb", name=f"msq{hc}")
                        nc.scalar.activation(sq[:], pm[hc], AF.Square)
                        nc.tensor.matmul(mss[:], wfi_sb[:, hc:hc + 1], sq[:],
                                         start=(hc == 0), stop=(hc == HC - 1))
                    r3_row = t1.tile([1, SL], F32, tag="row", name="r3_row")
                    nc.scalar.activation(r3_row[:], mss[:], AF.Sqrt,
                                         bias=eps128[0:1, :], scale=1.0 / H)
                    nc.vector.reciprocal_approx_fast(r3_row[:], r3_row[:])
                    r3_b = t1.tile([P, SL], F32, tag="bcast", name="r3_b")
                    nc.gpsimd.partition_broadcast(r3_b[:], r3_row[:], channels=P)
                    for hc in range(HC):
                        tm = t1.tile([P, SL], F32, tag="htmp", name=f"om{hc}")
                        nc.vector.tensor_mul(tm[:], pm[hc], r3_b[:])
                        out_c = t1.tile([P, SL], F32, tag="outc", name=f"oc{hc}",
                                        bufs=3)
                        nc.vector.tensor_add(out_c[:], tm[:], h2[:, hc, :])
                        nc.sync.dma_start(outt[:, hc, :], out_c[:])

    nc.compile()
    return nc


def _get_nc():
    if "nc" not in _CACHED:
        _CACHED["nc"] = _build()
    return _CACHED["nc"]


def _prep_inputs(hidden_states, cos, sin, wq, wk, wv, wo, q_norm_w, k_norm_w,
                 ln_in_w, ln_post_attn_w, ln_pre_ffn_w, ln_post_ffn_w,
                 wg, wu, wd):
    f = np.float32
    bf = ml_dtypes.bfloat16
    ct = np.ascontiguousarray

    hid = np.asarray(hidden_states, f)[0]            # [S, H]
    hT = ct(hid.T)                                   # [H, S]
    cosT = ct(np.asarray(cos, f)[0, 0].T)            # [D, S]
    sinT = ct(np.asarray(sin, f)[0, 0].T)

    g_in = 1.0 + np.asarray(ln_in_w, f)
    g_ffn = 1.0 + np.asarray(ln_pre_ffn_w, f)
    g_att = 1.0 + np.asarray(ln_post_attn_w, f)
    g_out = 1.0 + np.asarray(ln_post_ffn_w, f)

    def pmaj(a, nch):   # [nch*P, X] -> [P, nch, X]
        return ct(a.reshape(nch, P, a.shape[1]).transpose(1, 0, 2))

    def pmaj_sl(a, nch):  # [nch*P, S] -> [P, NSL, nch, 512]
        return ct(a.reshape(nch, P, NSL, 512).transpose(1, 2, 0, 3))

    wot_p = (np.asarray(wo, f).T * g_att[None, :])         .reshape(NH, DC, P, H).transpose(1, 0, 2, 3).reshape(H, H)

    shared = {
        "ht": pmaj_sl(hT, HC),
        "cost": pmaj_sl(cosT, DC),
        "sint": pmaj_sl(sinT, DC),
        "wqt": pmaj((np.asarray(wq, f) * g_in[None, :]).T, HC),
        "wkt": pmaj((np.asarray(wk, f) * g_in[None, :]).T, HC),
        "wvt": pmaj((np.asarray(wv, f) * g_in[None, :]).T, HC),
        "wot": pmaj(wot_p, HC).astype(bf),
        "wgt": pmaj((np.asarray(wg, f) * g_ffn[None, :]).T, HC).astype(bf),
        "wut": pmaj((np.asarray(wu, f) * g_ffn[None, :]).T, HC).astype(bf),
        "wdt": pmaj((np.asarray(wd, f).T * g_out[None, :]), IC).astype(bf),
        "qw1": ct((1.0 + np.asarray(q_norm_w, f)).reshape(DC, P).T),
        "kw1": ct((1.0 + np.asarray(k_norm_w, f)).reshape(DC, P).T),
        "wai": ct((g_att ** -2.0).reshape(HC, P).T.astype(bf)),
        "wfi": ct((g_out ** -2.0).reshape(HC, P).T.astype(bf)),
    }
    in_maps = []
    for c in range(NC):
        cols = slice(c * SL, (c + 1) * SL)
        m = dict(shared)
        m["hq"] = pmaj(hT[:, cols], HC)
        m["cosq"] = pmaj(cosT[:, cols], DC)
        m["sinq"] = pmaj(sinT[:, cols], DC)
        in_maps.append(m)
    return in_maps


def run(trace=False, tmpdir=None, **inputs):
    """Build (cached), run on 8 cores, reassemble. Returns (output, results)."""
    nc = _get_nc()
    in_maps = _prep_inputs(
        inputs["hidden_states"], inputs["cos"], inputs["sin"],
        inputs["wq"], inputs["wk"], inputs["wv"], inputs["wo"],
        inputs["q_norm_w"], inputs["k_norm_w"],
        inputs["ln_in_w"], inputs["ln_post_attn_w"],
        inputs["ln_pre_ffn_w"], inputs["ln_post_ffn_w"],
        inputs["wg"], inputs["wu"], inputs["wd"],
    )
    res = run_bass_kernel_spmd(nc, in_maps, list(range(NC)),
                               trace=trace, tmpdir=tmpdir)
    out = np.empty((S, H), np.float32)
    for c in range(NC):
        o = res.results[c]["outt"]            # [P, HC, SL]
        out[c * SL:(c + 1) * SL, :] = o.transpose(1, 0, 2).reshape(H, SL).T
    return out[None], res


def kernel(**inputs):
    out, _ = run(trace=False, **inputs)
    return out


# revision 15
# speedup vs baseline: 1.0937x; 1.0937x over previous
"""Trainium2 Bass kernel for nn_EncoderLayer_31825707664096.

Gemma-style encoder layer (RMSNorm + GQA attention w/ QK-norm + RoPE + GeGLU
MLP), batch=1, seq=2048, hidden=768, 3 heads x 256 head_dim, 1 KV head,
inter=1152, fp32.

Strategy: sequence-parallel over 8 cores, no collectives (cross-core sync
costs ~90us of launch skew here). Each core owns 256 query rows and
recomputes full K/V, streaming the hidden state through SBUF in 512-column
slices so compute starts ~3us in. All activations are feature-major.

Precision tiers: q/k score path float32r (TF32-like, full PE rate at
free>=256); exp output u and V are bf16 (linear averaging errors only);
O-projection and MLP run bf16 weights/activations. PSUM always fp32;
residual path fp32.

Folds (host or on-chip, all exact):
- (1+ln_in_w) into wq/wk/wv rows; (1+ln_pre_ffn_w) into wg/wu rows
- (1+ln_post_attn_w) into wo columns; the rmsnorm stats then reduce with
  1/(1+w)^2 weights so the rstd sees unscaled values. Same for wd.
- input-norm rstd (rin) cancels in q/k norm; for V it rides the PSUM->SBUF
  copy as a per-partition scale (column stats via transposed PE reduction)
- k-norm rstd (ck) is the per-partition scale of the softmax exp
- softmax runs unnormalized with a constant shift exp(s - C); the per-query
  normalizer is applied to the attention output (reciprocal + broadcast)
"""

from contextlib import ExitStack

import numpy as np
import ml_dtypes

import concourse.mybir as mybir
import concourse.tile as tile
from concourse import bacc
from concourse.bass_utils import run_bass_kernel_spmd

P = 128
S = 2048          # sequence length
H = 768           # hidden
D = 256           # head dim (also total KV width)
NH = 3            # query heads
I = 1152          # mlp intermediate
NC = 8            # cores
SL = S // NC      # 256 query rows per core
SC = SL // P      # 2
HC = H // P       # 6
DC = D // P       # 2
IC = I // P       # 9
KC = S // P       # 16 key chunks
NSL = S // 512    # 4 512-wide slices
EPS = 1e-6
C_SHIFT = 30.0    # exp(s - C_SHIFT): keeps unnormalized softmax in fp32 range

F32 = mybir.dt.float32
F32R = mybir.dt.float32r
BF16 = mybir.dt.bfloat16
MUL = mybir.AluOpType.mult
AF = mybir.ActivationFunctionType

_CACHED = {}


def _build():
    nc = bacc.Bacc("TRN2", target_bir_lowering=False, debug=False, num_devices=NC)

    # ---- DRAM I/O ----
    ht = nc.dram_tensor("ht", [P, NSL, HC, 512], F32R, kind="ExternalInput").ap()
    hq = nc.dram_tensor("hq", [P, HC, SL], F32R, kind="ExternalInput").ap()
    cost = nc.dram_tensor("cost", [P, NSL, DC, 512], F32, kind="ExternalInput").ap()
    sint = nc.dram_tensor("sint", [P, NSL, DC, 512], F32, kind="ExternalInput").ap()
    cosq = nc.dram_tensor("cosq", [P, DC, SL], F32, kind="ExternalInput").ap()
    sinq = nc.dram_tensor("sinq", [P, DC, SL], F32, kind="ExternalInput").ap()
    wqt = nc.dram_tensor("wqt", [P, HC, H], F32R, kind="ExternalInput").ap()
    wkt = nc.dram_tensor("wkt", [P, HC, D], F32R, kind="ExternalInput").ap()
    wvt = nc.dram_tensor("wvt", [P, HC, D], F32R, kind="ExternalInput").ap()
    wot = nc.dram_tensor("wot", [P, HC, H], BF16, kind="ExternalInput").ap()
    wgt = nc.dram_tensor("wgt", [P, HC, I], BF16, kind="ExternalInput").ap()
    wut = nc.dram_tensor("wut", [P, HC, I], BF16, kind="ExternalInput").ap()
    wdt = nc.dram_tensor("wdt", [P, IC, H], BF16, kind="ExternalInput").ap()
    qw1 = nc.dram_tensor("qw1", [P, DC], F32, kind="ExternalInput").ap()   # 1+q_norm_w
    kw1 = nc.dram_tensor("kw1", [P, DC], F32, kind="ExternalInput").ap()   # 1+k_norm_w
    wai = nc.dram_tensor("wai", [P, HC], BF16, kind="ExternalInput").ap()  # (1+ln_post_attn)^-2
    wfi = nc.dram_tensor("wfi", [P, HC], BF16, kind="ExternalInput").ap()  # (1+ln_post_ffn)^-2
    outt = nc.dram_tensor("outt", [P, HC, SL], F32, kind="ExternalOutput").ap()

    def cp(ap2d):  # [(c p), x] -> [p, c, x]
        return ap2d.rearrange("(c p) x -> p c x", p=P)

    def f32(ap):
        return ap.bitcast(F32)

    with tile.TileContext(nc) as tc:
        with (
            tc.tile_pool(name="persist", bufs=1) as pp,
            tc.tile_pool(name="t1", bufs=2) as t1,
            tc.tile_pool(name="pmm", bufs=3, space="PSUM") as pmm,
            tc.tile_pool(name="pst", bufs=1, space="PSUM") as pst,
        ):
            # ---- small constants ----
            ones = pp.tile([P, 1], F32, tag="ones")
            nc.vector.memset(ones[:], 1.0)
            ones_r = ones[:].bitcast(F32R)
            ones_b = pp.tile([P, 1], BF16, tag="ones_b")
            nc.vector.memset(ones_b[:], 1.0)
            ones2b = pp.tile([P, 2], BF16, tag="ones2b")
            nc.vector.memset(ones2b[:], 1.0)
            eps128 = pp.tile([P, 1], F32, tag="eps128")
            nc.vector.memset(eps128[:], EPS)
            biasC = pp.tile([P, 1], F32, tag="biasC")
            nc.vector.memset(biasC[:], -C_SHIFT)
            qw1_sb = pp.tile([P, DC], F32, tag="qw1")
            nc.sync.dma_start(qw1_sb[:], qw1)
            kw1_sb = pp.tile([P, DC], F32, tag="kw1")
            nc.sync.dma_start(kw1_sb[:], kw1)
            wai_sb = pp.tile([P, HC], BF16, tag="wai")
            nc.sync.dma_start(wai_sb[:], wai)
            wfi_sb = pp.tile([P, HC], BF16, tag="wfi")
            nc.sync.dma_start(wfi_sb[:], wfi)

            # ---- startup-critical loads ----
            hq_sb = pp.tile([P, HC, SL], F32R, tag="hq")
            nc.sync.dma_start(hq_sb[:], hq)
            es = ExitStack()
            wp1 = es.enter_context(tc.tile_pool(name="wp1", bufs=1))
            wk_sb = wp1.tile([P, HC, D], F32R, tag="wk")
            nc.scalar.dma_start(wk_sb[:], wkt)
            wv_sb = wp1.tile([P, HC, D], F32R, tag="wv")
            nc.scalar.dma_start(wv_sb[:], wvt)

            # persistent activations
            qt_f = pp.tile([P, HC, SL], F32R, tag="qtf")
            kt_sb = pp.tile([P, DC, S], F32R, tag="ktf")
            v_sb = pp.tile([P, KC, D], BF16, tag="v")
            at_f = pp.tile([P, DC, NH, SL], BF16, tag="atf")
            h2 = pp.tile([P, HC, SL], F32, tag="h2")
            h2n = pp.tile([P, HC, SL], BF16, tag="h2n")
            rin_col = pp.tile([P, KC], F32, tag="rin")
            ck_col = pp.tile([P, KC], F32, tag="ck")

            # =====================================================
            # K + stats over full S, streamed in 512-col slices
            # =====================================================
            esl = ExitStack()
            slp = esl.enter_context(tc.tile_pool(name="slp", bufs=2))
            psc = esl.enter_context(tc.tile_pool(name="psc", bufs=1, space="PSUM"))
            iss_t = psc.tile([P, KC, 2], F32, tag="ips", name="iss")
            ck_t = psc.tile([P, KC, 2], F32, tag="cps", name="cks")
            iss_ps = iss_t[:]
            ck_ps = ck_t[:]

            for sl in range(NSL):
                sls = slice(sl * 512, (sl + 1) * 512)
                hsl = slp.tile([P, HC, 512], F32R, tag="hsl", name=f"hsl{sl}")
                nc.sync.dma_start(hsl[:], ht[:, sl, :, :])
                cs = slp.tile([P, DC, 512], F32, tag="cs", name=f"cs{sl}")
                nc.scalar.dma_start(cs[:], cost[:, sl, :, :])
                sn = slp.tile([P, DC, 512], F32, tag="sn", name=f"sn{sl}")
                nc.scalar.dma_start(sn[:], sint[:, sl, :, :])

                # K projection for this slice
                pk = [psc.tile([P, 512], F32, tag="pk", name=f"pk{sl}_{d_}", bufs=2)
                      for d_ in range(DC)]
                for d in range(DC):
                    for kc in range(HC):
                        nc.tensor.matmul(
                            pk[d][:],
                            wk_sb[:, kc, d * P:(d + 1) * P],
                            hsl[:, kc, :],
                            start=(kc == 0), stop=(kc == HC - 1),
                        )
                # k-norm stats (column orientation)
                ksq = []
                for d in range(DC):
                    sq = t1.tile([P, 512], BF16, tag="ksq", name=f"ksq{sl}_{d}")
                    nc.scalar.activation(sq[:], pk[d][:], AF.Square)
                    ksq.append(sq)
                for c4 in range(4):
                    ksc = 4 * sl + c4
                    for d in range(DC):
                        nc.tensor.matmul(
                            ck_ps[:, ksc, :],
                            ksq[d][:, c4 * P:(c4 + 1) * P],
                            ones2b[:],
                            start=(d == 0), stop=(d == DC - 1),
                        )
                # rope -> kt
                t0 = t1.tile([P, 512], F32, tag="rA", name=f"krA{sl}")
                tb = t1.tile([P, 512], F32, tag="rB", name=f"krB{sl}")
                nc.vector.scalar_tensor_tensor(
                    t0[:], pk[0][:], kw1_sb[:, 0:1], cs[:, 0, :], MUL, MUL)
                nc.vector.scalar_tensor_tensor(
                    tb[:], pk[1][:], kw1_sb[:, 1:2], sn[:, 0, :], MUL, MUL)
                nc.vector.tensor_sub(kt_sb[:, 0, sls], t0[:], tb[:])
                t2 = t1.tile([P, 512], F32, tag="rA", name=f"krC{sl}")
                t3 = t1.tile([P, 512], F32, tag="rB", name=f"krD{sl}")
                nc.vector.scalar_tensor_tensor(
                    t2[:], pk[1][:], kw1_sb[:, 1:2], cs[:, 1, :], MUL, MUL)
                nc.vector.scalar_tensor_tensor(
                    t3[:], pk[0][:], kw1_sb[:, 0:1], sn[:, 1, :], MUL, MUL)
                nc.vector.tensor_add(kt_sb[:, 1, sls], t2[:], t3[:])

                # input sumsq (column orientation) for V scale
                isq_t = []
                for kc in range(HC):
                    isq = t1.tile([P, 512], BF16, tag="isq", name=f"isq{sl}_{kc}",
                                  bufs=HC)
                    nc.vector.tensor_mul(isq[:], f32(hsl[:, kc, :]),
                                         f32(hsl[:, kc, :]))
                    isq_t.append(isq)
                for c4 in range(4):
                    for kc in range(HC):
                        nc.tensor.matmul(
                            iss_ps[:, 4 * sl + c4, :],
                            isq_t[kc][:, c4 * P:(c4 + 1) * P],
                            ones2b[:],
                            start=(kc == 0), stop=(kc == HC - 1),
                        )

                # V projection for this slice (raw; rin applied afterwards)
                for c4 in range(4):
                    sc = 4 * sl + c4
                    pv = pmm.tile([P, D], F32, tag="mm", name=f"pv{sc}")
                    for kc in range(HC):
                        nc.tensor.matmul(
                            pv[:],
                            hsl[:, kc, c4 * P:(c4 + 1) * P],
                            wv_sb[:, kc, :],
                            start=(kc == 0), stop=(kc == HC - 1),
                        )
                    nc.scalar.copy(v_sb[:, sc, :], pv[:])

            # rstd columns: rin [P, KC], ck [P, KC]
            nc.scalar.activation(
                rin_col[:],
                iss_ps[:, :, 0:1].rearrange("p a b -> p (a b)"),
                AF.Sqrt, bias=eps128[:], scale=1.0 / H)
            nc.vector.reciprocal_approx_fast(rin_col[:], rin_col[:])
            nc.scalar.activation(
                ck_col[:],
                ck_ps[:, :, 0:1].rearrange("p a b -> p (a b)"),
                AF.Sqrt, bias=eps128[:], scale=1.0 / D)
            nc.vector.reciprocal_approx_fast(ck_col[:], ck_col[:])

            esl.close()  # free slice ring + slice psum

            # =====================================================
            # Q projection + q-norm + rope
            # =====================================================
            cosq_sb = pp.tile([P, DC, SL], F32, tag="cosq")
            nc.sync.dma_start(cosq_sb[:], cosq)
            sinq_sb = pp.tile([P, DC, SL], F32, tag="sinq")
            nc.sync.dma_start(sinq_sb[:], sinq)
            wq_sb = wp1.tile([P, HC, H], F32R, tag="wq")
            nc.sync.dma_start(wq_sb[:], wqt)
            wo_sb = pp.tile([P, HC, H], BF16, tag="wo")
            nc.sync.dma_start(wo_sb[:], wot)
            wg_sb = pp.tile([P, HC, I], BF16, tag="wg")
            nc.sync.dma_start(wg_sb[:], wgt)
            wu_sb = pp.tile([P, HC, I], BF16, tag="wu")
            nc.sync.dma_start(wu_sb[:], wut)
            wd_sb = pp.tile([P, IC, H], BF16, tag="wd")
            nc.sync.dma_start(wd_sb[:], wdt)

            for h in range(NH):
                pq = [pmm.tile([P, SL], F32, tag="mm", name=f"pq{h}_{d_}")
                      for d_ in range(DC)]
                for d in range(DC):
                    oc = 2 * h + d
                    for kc in range(HC):
                        nc.tensor.matmul(
                            pq[d][:],
                            wq_sb[:, kc, oc * P:(oc + 1) * P],
                            hq_sb[:, kc, :],
                            start=(kc == 0), stop=(kc == HC - 1),
                        )
                qss = pst.tile([1, SL], F32, tag="st1", name=f"qss{h}")
                for d in range(DC):
                    sq = t1.tile([P, SL], F32R, tag="sq", name=f"qsq{h}_{d}")
                    nc.scalar.activation(sq[:], pq[d][:], AF.Square)
                    nc.tensor.matmul(qss[:], ones_r, sq[:],
                                     start=(d == 0), stop=(d == DC - 1))
                cq_row = t1.tile([1, SL], F32, tag="row", name=f"cqr{h}")
                nc.scalar.activation(cq_row[:], qss[:], AF.Sqrt,
                                     bias=eps128[0:1, :], scale=1.0 / D)
                nc.vector.reciprocal_approx_fast(cq_row[:], cq_row[:])
                cq_b = t1.tile([P, SL], F32, tag="bcast", name=f"cqb{h}")
                nc.gpsimd.partition_broadcast(cq_b[:], cq_row[:], channels=P)
                t0 = t1.tile([P, SL], F32, tag="rA", name=f"rA{h}")
                tb = t1.tile([P, SL], F32, tag="rB", name=f"rB{h}")
                nc.vector.scalar_tensor_tensor(
                    t0[:], pq[0][:], qw1_sb[:, 0:1], cosq_sb[:, 0, :], MUL, MUL)
                nc.vector.scalar_tensor_tensor(
                    tb[:], pq[1][:], qw1_sb[:, 1:2], sinq_sb[:, 0, :], MUL, MUL)
                nc.vector.tensor_sub(t0[:], t0[:], tb[:])
                nc.vector.tensor_mul(qt_f[:, 2 * h, :], t0[:], cq_b[:])
                t2 = t1.tile([P, SL], F32, tag="rA", name=f"rC{h}")
                t3 = t1.tile([P, SL], F32, tag="rB", name=f"rD{h}")
                nc.vector.scalar_tensor_tensor(
                    t2[:], pq[1][:], qw1_sb[:, 1:2], cosq_sb[:, 1, :], MUL, MUL)
                nc.vector.scalar_tensor_tensor(
                    t3[:], pq[0][:], qw1_sb[:, 0:1], sinq_sb[:, 1, :], MUL, MUL)
                nc.vector.tensor_add(t2[:], t2[:], t3[:])
                nc.vector.tensor_mul(qt_f[:, 2 * h + 1, :], t2[:], cq_b[:])

            es.close()  # free wq/wk/wv

            # =====================================================
            # attention: scoresT -> exp(scale=ck) -> den -> A^T V
            # =====================================================
            with tc.tile_pool(name="t2", bufs=2) as t2p:
                u3 = t2p.tile([P, NH, KC, SL], BF16, tag="u3", name="u3", bufs=1)
                den_b = t2p.tile([P, NH, SL], F32, tag="denb", name="den_b", bufs=1)
                esa = ExitStack()
                psa = esa.enter_context(
                    tc.tile_pool(name="psa", bufs=1, space="PSUM"))
                # heads 0+1 paired into 512-wide matmuls; head 2 narrow
                for ksc in range(KC):
                    ps2 = psa.tile([P, 2, SL], F32, tag="mm2", name=f"ps2_{ksc}",
                                   bufs=2)
                    for d in range(DC):
                        nc.tensor.matmul(
                            ps2[:],
                            kt_sb[:, d, ksc * P:(ksc + 1) * P],
                            qt_f[:, d:3 + d:2, :],
                            start=(d == 0), stop=(d == DC - 1),
                        )
                    nc.scalar.activation(u3[:, 0:2, ksc, :], ps2[:], AF.Exp,
                                         bias=biasC[:],
                                         scale=ck_col[:, ksc:ksc + 1])
                    ps_ = pmm.tile([P, SL], F32, tag="mm", name=f"ps{ksc}")
                    for d in range(DC):
                        nc.tensor.matmul(
                            ps_[:],
                            kt_sb[:, d, ksc * P:(ksc + 1) * P],
                            qt_f[:, 4 + d, :],
                            start=(d == 0), stop=(d == DC - 1),
                        )
                    nc.scalar.activation(u3[:, 2, ksc, :], ps_[:], AF.Exp,
                                         bias=biasC[:],
                                         scale=ck_col[:, ksc:ksc + 1])

                # apply rin to V (off the exp critical path)
                for sc in range(KC):
                    nc.scalar.mul(v_sb[:, sc, :], v_sb[:, sc, :],
                                  rin_col[:, sc:sc + 1])

                den2 = psa.tile([1, 2, SL], F32, tag="dn2", name="den01")
                for ksc in range(KC):
                    nc.tensor.matmul(
                        den2[:], ones_b[:], u3[:, 0:2, ksc, :],
                        start=(ksc == 0), stop=(ksc == KC - 1))
                den2_row = t1.tile([1, 2, SL], F32, tag="row2", name="denr01")
                nc.vector.reciprocal_approx_fast(
                    den2_row[:].rearrange("o a s -> o (a s)"),
                    den2[:].rearrange("o a s -> o (a s)"))
                for h in range(2):
                    nc.gpsimd.partition_broadcast(den_b[:, h, :],
                                                  den2_row[:, h, :], channels=P)
                den = pst.tile([1, SL], F32, tag="st1", name="den2")
                for ksc in range(KC):
                    nc.tensor.matmul(den[:], ones_b[:], u3[:, 2, ksc, :],
                                     start=(ksc == 0), stop=(ksc == KC - 1))
                den_row = t1.tile([1, SL], F32, tag="row", name="denr2")
                nc.vector.reciprocal_approx_fast(den_row[:], den[:])
                nc.gpsimd.partition_broadcast(den_b[:, 2, :], den_row[:],
                                              channels=P)

                for d in range(DC):
                    pn2 = psa.tile([P, 2, SL], F32, tag="mm2", name=f"pn2_{d}",
                                   bufs=2)
                    for ksc in range(KC):
                        nc.tensor.matmul(
                            pn2[:],
                            v_sb[:, ksc, d * P:(d + 1) * P],
                            u3[:, 0:2, ksc, :],
                            start=(ksc == 0), stop=(ksc == KC - 1),
                        )
                    nc.vector.tensor_mul(at_f[:, d, 0:2, :], pn2[:],
                                         den_b[:, 0:2, :])
                    pn = pmm.tile([P, SL], F32, tag="mm", name=f"pn{d}")
                    for ksc in range(KC):
                        nc.tensor.matmul(
                            pn[:],
                            v_sb[:, ksc, d * P:(d + 1) * P],
                            u3[:, 2, ksc, :],
                            start=(ksc == 0), stop=(ksc == KC - 1),
                        )
                    nc.vector.tensor_mul(at_f[:, d, 2, :], pn[:],
                                         den_b[:, 2, :])
                esa.close()
                at_v = at_f[:].rearrange("p d h s -> p (d h) s")

                # =====================================================
                # wo projection + post-attn rmsnorm + residual
                # =====================================================
                with tc.tile_pool(name="pho", bufs=1, space="PSUM") as pho:
                    ppo2 = [pho.tile([P, 2, SL], F32, tag=f"po{j}", name=f"pp{j}")
                            for j in range(HC // 2)]
                    ppo = [ppo2[j][:, i, :] for j in range(HC // 2) for i in range(2)]
                    pss = pst.tile([1, SL], F32, tag="st1", name="pss")
                    for hc in range(HC):
                        for oc in range(HC):
                            nc.tensor.matmul(
                                ppo[hc],
                                wo_sb[:, oc, hc * P:(hc + 1) * P],
                                at_v[:, oc, :],
                                start=(oc == 0), stop=(oc == HC - 1),
                            )
                        sq = t1.tile([P, SL], BF16, tag="sqb", name=f"psq{hc}")
                        nc.scalar.activation(sq[:], ppo[hc], AF.Square)
                        nc.tensor.matmul(pss[:], wai_sb[:, hc:hc + 1], sq[:],
                                         start=(hc == 0), stop=(hc == HC - 1))
                    ra_row = t1.tile([1, SL], F32, tag="row", name="ra_row")
                    nc.scalar.activation(ra_row[:], pss[:], AF.Sqrt,
                                         bias=eps128[0:1, :], scale=1.0 / H)
                    nc.vector.reciprocal_approx_fast(ra_row[:], ra_row[:])
                    ra_b = t1.tile([P, SL], F32, tag="bcast", name="ra_b")
                    nc.gpsimd.partition_broadcast(ra_b[:], ra_row[:], channels=P)
                    for hc in range(HC):
                        tm = t1.tile([P, SL], F32, tag="htmp", name=f"hm{hc}")
                        nc.vector.tensor_mul(tm[:], ppo[hc], ra_b[:])
                        nc.vector.tensor_add(h2[:, hc, :], tm[:],
                                             f32(hq_sb[:, hc, :]))

                # =====================================================
                # pre-FFN rmsnorm
                # =====================================================
                fss = pst.tile([1, SL], F32, tag="st1", name="fss")
                for hc in range(HC):
                    sq = t1.tile([P, SL], BF16, tag="sqb", name=f"fsq{hc}")
                    nc.vector.tensor_mul(sq[:], h2[:, hc, :], h2[:, hc, :])
                    nc.tensor.matmul(fss[:], ones_b[:], sq[:],
                                     start=(hc == 0), stop=(hc == HC - 1))
                r2_row = t1.tile([1, SL], F32, tag="row", name="r2_row")
                nc.scalar.activation(r2_row[:], fss[:], AF.Sqrt,
                                     bias=eps128[0:1, :], scale=1.0 / H)
                nc.vector.reciprocal_approx_fast(r2_row[:], r2_row[:])
                r2_b = t1.tile([P, SL], F32, tag="bcast", name="r2_b")
                nc.gpsimd.partition_broadcast(r2_b[:], r2_row[:], channels=P)
                for hc in range(HC):
                    nc.vector.tensor_mul(h2n[:, hc, :], h2[:, hc, :], r2_b[:])

                # =====================================================
                # MLP: gate/up -> gelu_tanh * up -> down + post-ffn norm
                # =====================================================
                gall = t2p.tile([P, IC, SL], BF16, tag="gall", name="gall", bufs=1)
                act = t2p.tile([P, IC, SL], BF16, tag="act", name="act", bufs=1)
                for ic in range(IC):
                    pg = pmm.tile([P, SL], F32, tag="mm", name=f"pg{ic}")
                    for kc in range(HC):
                        nc.tensor.matmul(pg[:], wg_sb[:, kc, ic * P:(ic + 1) * P],
                                         h2n[:, kc, :],
                                         start=(kc == 0), stop=(kc == HC - 1))
                    nc.scalar.activation(gall[:, ic, :], pg[:], AF.Gelu_apprx_tanh)
                    pu = pmm.tile([P, SL], F32, tag="mm", name=f"pu{ic}")
                    for kc in range(HC):
                        nc.tensor.matmul(pu[:], wu_sb[:, kc, ic * P:(ic + 1) * P],
                                         h2n[:, kc, :],
                                         start=(kc == 0), stop=(kc == HC - 1))
                    nc.vector.tensor_mul(act[:, ic, :], gall[:, ic, :], pu[:])

                with tc.tile_pool(name="phd", bufs=1, space="PSUM") as phd:
                    pm2 = [phd.tile([P, 2, SL], F32, tag=f"md{j}", name=f"pm{j}")
                           for j in range(HC // 2)]
                    pm = [pm2[j][:, i, :] for j in range(HC // 2) for i in range(2)]
                    mss = pst.tile([1, SL], F32, tag="st1", name="mss")
                    for hc in range(HC):
                        for ic in range(IC):
                            nc.tensor.matmul(pm[hc],
                                             wd_sb[:, ic, hc * P:(hc + 1) * P],
                                             act[:, ic, :],
                                             start=(ic == 0), stop=(ic == IC - 1))
                        sq = t1.tile([P, SL], BF16, tag="sqb", name=f"msq{hc}")
                        nc.scalar.activation(sq[:], pm[hc], AF.Square)
                        nc.tensor.matmul(mss[:], wfi_sb[:, hc:hc + 1], sq[:],
                                         start=(hc == 0), stop=(hc == HC - 1))
                    r3_row = t1.tile([1, SL], F32, tag="row", name="r3_row")
                    nc.scalar.activation(r3_row[:], mss[:], AF.Sqrt,
                                         bias=eps128[0:1, :], scale=1.0 / H)
                    nc.vector.reciprocal_approx_fast(r3_row[:], r3_row[:])
                    r3_b = t1.tile([P, SL], F32, tag="bcast", name="r3_b")
                    nc.gpsimd.partition_broadcast(r3_b[:], r3_row[:], channels=P)
                    for hc in range(HC):
                        tm = t1.tile([P, SL], F32, tag="htmp", name=f"om{hc}")
                        nc.vector.tensor_mul(tm[:], pm[hc], r3_b[:])
                        out_c = t1.tile([P, SL], F32, tag="outc", name=f"oc{hc}",
                                        bufs=3)
                        nc.vector.tensor_add(out_c[:], tm[:], h2[:, hc, :])
                        nc.sync.dma_start(outt[:, hc, :], out_c[:])

    nc.compile()
    return nc


def _get_nc():
    if "nc" not in _CACHED:
        _CACHED["nc"] = _build()
    return _CACHED["nc"]


def _prep_inputs(hidden_states, cos, sin, wq, wk, wv, wo, q_norm_w, k_norm_w,
                 ln_in_w, ln_post_attn_w, ln_pre_ffn_w, ln_post_ffn_w,
                 wg, wu, wd):
    f = np.float32
    bf = ml_dtypes.bfloat16
    ct = np.ascontiguousarray

    hid = np.asarray(hidden_states, f)[0]            # [S, H]
    hT = ct(hid.T)                                   # [H, S]
    cosT = ct(np.asarray(cos, f)[0, 0].T)            # [D, S]
    sinT = ct(np.asarray(sin, f)[0, 0].T)

    g_in = 1.0 + np.asarray(ln_in_w, f)
    g_ffn = 1.0 + np.asarray(ln_pre_ffn_w, f)
    g_att = 1.0 + np.asarray(ln_post_attn_w, f)
    g_out = 1.0 + np.asarray(ln_post_ffn_w, f)

    def pmaj(a, nch):   # [nch*P, X] -> [P, nch, X]
        return ct(a.reshape(nch, P, a.shape[1]).transpose(1, 0, 2))

    def pmaj_sl(a, nch):  # [nch*P, S] -> [P, NSL, nch, 512]
        return ct(a.reshape(nch, P, NSL, 512).transpose(1, 2, 0, 3))

    wot_p = (np.asarray(wo, f).T * g_att[None, :])         .reshape(NH, DC, P, H).transpose(1, 0, 2, 3).reshape(H, H)

    shared = {
        "ht": pmaj_sl(hT, HC),
        "cost": pmaj_sl(cosT, DC),
        "sint": pmaj_sl(sinT, DC),
        "wqt": pmaj((np.asarray(wq, f) * g_in[None, :]).T, HC),
        "wkt": pmaj((np.asarray(wk, f) * g_in[None, :]).T, HC),
        "wvt": pmaj((np.asarray(wv, f) * g_in[None, :]).T, HC),
        "wot": pmaj(wot_p, HC).astype(bf),
        "wgt": pmaj((np.asarray(wg, f) * g_ffn[None, :]).T, HC).astype(bf),
        "wut": pmaj((np.asarray(wu, f) * g_ffn[None, :]).T, HC).astype(bf),
        "wdt": pmaj((np.asarray(wd, f).T * g_out[None, :]), IC).astype(bf),
        "qw1": ct((1.0 + np.asarray(q_norm_w, f)).reshape(DC, P).T),
        "kw1": ct((1.0 + np.asarray(k_norm_w, f)).reshape(DC, P).T),
        "wai": ct((g_att ** -2.0).reshape(HC, P).T.astype(bf)),
        "wfi": ct((g_out ** -2.0).reshape(HC, P).T.astype(bf)),
    }
    in_maps = []
    for c in range(NC):
        cols = slice(c * SL, (c + 1) * SL)
        m = dict(shared)
        m["hq"] = pmaj(hT[:, cols], HC)
        m["cosq"] = pmaj(cosT[:, cols], DC)
        m["sinq"] = pmaj(sinT[:, cols], DC)
        in_maps.append(m)
    return in_maps


def run(trace=False, tmpdir=None, **inputs):
    """Build (cached), run on 8 cores, reassemble. Returns (output, results)."""
    nc = _get_nc()
    in_maps = _prep_inputs(
        inputs["hidden_states"], inputs["cos"], inputs["sin"],
        inputs["wq"], inputs["wk"], inputs["wv"], inputs["wo"],
        inputs["q_norm_w"], inputs["k_norm_w"],
        inputs["ln_in_w"], inputs["ln_post_attn_w"],
        inputs["ln_pre_ffn_w"], inputs["ln_post_ffn_w"],
        inputs["wg"], inputs["wu"], inputs["wd"],
    )
    res = run_bass_kernel_spmd(nc, in_maps, list(range(NC)),
                               trace=trace, tmpdir=tmpdir)
    out = np.empty((S, H), np.float32)
    for c in range(NC):
        o = res.results[c]["outt"]            # [P, HC, SL]
        out[c * SL:(c + 1) * SL, :] = o.transpose(1, 0, 2).reshape(H, SL).T
    return out[None], res


def kernel(**inputs):
    out, _ = run(trace=False, **inputs)
    return out
